# revision 3
# baseline (speedup 1.0000x reference)
"""Trainium2 Bass kernel for nn_DifferentialFlashAttention.

Computation (per token t, fully parallel over the B*N = 32768 tokens):
  qkv = x @ W_attn.T ; split into q, k, v
  q/k split per head into two sub-vectors (q1, q2 / k1, k2) of 32 dims
  S_s[i,j] = q_s[i] . k_s[j] / sqrt(32)   (attention over the 12 HEADS)
  A_s = softmax_j(S_s); O_s[i] = sum_j A_s[i,j] * v[j]
  y = (O_1 - lam_full * O_2) * (1 - LAMBDA_INIT);  out = y @ W_out.T

Sharding: data-parallel over tokens across 8 NeuronCores (4096 tokens each).
Per core: tokens-on-partitions layout, 32 tiles of 128 tokens.

V2 design (all-fp16 datapath, PSUM accumulation in fp32):
  - GEMM1/GEMM2 fp16 operands on TensorE (stationary = transposed
    activation chunks, moving = pre-transposed fp16 weights in SBUF).
  - Per-token head-attention on VectorE in fp16 with every heavy
    tensor_tensor op shaped for the 2x DVE perf mode (innermost dims of
    both operands are real stride-1 runs, 4B-aligned; broadcasts only on
    outer dims).  Softmax combine y = (A1 - lam*A2).v folded before the
    A.V contraction so there is ONE product pass over (i,c,j).
  - Exp and all PSUM->SBUF copies on ScalarE.
  - The combine scale (1-LAMBDA_INIT) is folded into W_out on the host;
    -lam_full is folded into the sub-2 softmax reciprocal.
"""

import math
import sys

import numpy as np

if "/opt/trn_rl_repo" not in sys.path:
    sys.path.insert(0, "/opt/trn_rl_repo")

N_HEAD = 12
N_EMBD = 768
HEAD_DIM = 32  # per-sub head dim
DEPTH = 12
LAMBDA_INIT = 0.8 - 0.6 * math.exp(-0.3 * DEPTH)
B, N, D = 4, 8192, 768
N_CORES = 8
TOK_TOTAL = B * N
TOK_PER_CORE = TOK_TOTAL // N_CORES  # 4096
TILE_T = 128
KC = D // 128  # 6 contraction chunks
SCALE = 1.0 / math.sqrt(HEAD_DIM)

_PROG_CACHE = {}


def _build_program(ntok, nrep=1):
    import concourse.bacc as bacc
    import concourse.tile as tile
    from concourse import mybir
    from concourse.masks import make_identity

    f32 = mybir.dt.float32
    fp16 = mybir.dt.float16
    Exp = mybir.ActivationFunctionType.Exp
    X = mybir.AxisListType.X
    add = mybir.AluOpType.add
    mult = mybir.AluOpType.mult

    ntiles = ntok // TILE_T

    nc = bacc.Bacc(
        "TRN2", target_bir_lowering=False, debug=False, num_devices=N_CORES
    )
    # x, host-packed fp16 so each partition's per-tile data is one contiguous
    # run: xH[p, tile, c, ti] = x[tile*128+ti, c*128+p]
    xH_d = nc.dram_tensor(
        "xH", [128, ntok // TILE_T, KC, TILE_T], fp16, kind="ExternalInput"
    ).ap()
    waT_d = nc.dram_tensor("waT", [D, 3 * D], fp16, kind="ExternalInput").ap()
    woT_d = nc.dram_tensor("woT", [D, D], fp16, kind="ExternalInput").ap()
    nlam_d = nc.dram_tensor("nlam", [1, 1], f32, kind="ExternalInput").ap()
    out_d = nc.dram_tensor("out", [ntok, D], fp16, kind="ExternalOutput").ap()

    with tile.TileContext(nc) as tc:
        from contextlib import ExitStack

        with ExitStack() as ctx:
            singles = ctx.enter_context(tc.tile_pool(name="singles", bufs=1))

            # ---- persistent weights / constants ----
            # weights split per output-chunk into separate tiles so tile 0's
            # first GEMM chunk only waits on its own slice of the preamble
            WA_CH = [(ob * 512, min(512, 3 * D - ob * 512)) for ob in range(5)]
            WO_CH = [(0, 512), (512, 256)]
            waT_r = waT_d.rearrange("(c p) o -> p c o", p=128)
            woT_r = woT_d.rearrange("(c p) o -> p c o", p=128)
            wa_sbs = [
                singles.tile([128, KC, ow], fp16, name=f"wa{ob}", tag=f"wa{ob}")
                for ob, (o0, ow) in enumerate(WA_CH)
            ]
            wo_sbs = [
                singles.tile([128, KC, ow], fp16, name=f"wo{ob}", tag=f"wo{ob}")
                for ob, (o0, ow) in enumerate(WO_CH)
            ]
            for (o0, ow), wa_t in zip(WA_CH, wa_sbs):
                nc.sync.dma_start(out=wa_t[:], in_=waT_r[:, :, o0 : o0 + ow])
            for (o0, ow), wo_t in zip(WO_CH, wo_sbs):
                nc.sync.dma_start(out=wo_t[:], in_=woT_r[:, :, o0 : o0 + ow])
            ident = singles.tile([128, 128], fp16)
            make_identity(nc, ident[:])
            nlam_sb = singles.tile([128, 1], f32)
            nc.gpsimd.dma_start(out=nlam_sb[:], in_=nlam_d.to_broadcast((128, 1)))

            xpool = ctx.enter_context(tc.tile_pool(name="xt", bufs=3))
            qkvpool = ctx.enter_context(tc.tile_pool(name="qkv", bufs=2))
            ppool = ctx.enter_context(tc.tile_pool(name="prod", bufs=1))
            tpool = ctx.enter_context(tc.tile_pool(name="tree", bufs=2))
            popool = ctx.enter_context(tc.tile_pool(name="oprod", bufs=1))
            smalls = ctx.enter_context(tc.tile_pool(name="smalls", bufs=2))
            ypool = ctx.enter_context(tc.tile_pool(name="y", bufs=2))
            opool = ctx.enter_context(tc.tile_pool(name="o2", bufs=2))
            ps_g1 = ctx.enter_context(tc.tile_pool(name="ps1", bufs=3, space="PSUM"))
            ps_tr = ctx.enter_context(tc.tile_pool(name="pstr", bufs=2, space="PSUM"))
            ps_g2 = ctx.enter_context(tc.tile_pool(name="ps2", bufs=2, space="PSUM"))

            def emit_tile(it):
                t0 = it * TILE_T
                # ---- load x^T tile (stationary chunks for GEMM1) ----
                xt = xpool.tile([128, KC, TILE_T], fp16, tag="xt")
                nc.sync.dma_start(out=xt[:], in_=xH_d[:, it, :, :])

                # ---- GEMM1: qkv[t, 0:2304] ----
                qkv = qkvpool.tile([128, 3 * D], fp16)
                for ob, (o0, ow) in enumerate(WA_CH):
                    ps = ps_g1.tile([128, 512], f32)
                    for c in range(KC):
                        nc.tensor.matmul(
                            ps[:, :ow],
                            xt[:, c, :],
                            wa_sbs[ob][:, c, :],
                            start=(c == 0),
                            stop=(c == KC - 1),
                        )
                    nc.scalar.copy(qkv[:, o0 : o0 + ow], ps[:, :ow])

                # ---- attention over heads, per token ----
                # S[s,i,j] = sum_d q[s,i,d]*k[s,j,d]; both subs in one op
                qv = (
                    qkv[:, 0:D]
                    .rearrange("p (i s d) -> p s i d", i=12, s=2)
                    .unsqueeze(3)
                    .broadcast_to((128, 2, 12, 12, 32))
                )
                kv = (
                    qkv[:, D : 2 * D]
                    .rearrange("p (j s d) -> p s j d", j=12, s=2)
                    .unsqueeze(2)
                    .broadcast_to((128, 2, 12, 12, 32))
                )
                P = ppool.tile([128, 288, 32], fp16, tag="P")
                nc.vector.tensor_mul(
                    P[:].rearrange("p (s i j) d -> p s i j d", s=2, i=12), qv, kv
                )
                # tree-sum over d (fp16 tensor_tensor runs 2x; tensor_reduce 1x)
                T1 = tpool.tile([128, 288, 16], fp16, tag="T1")
                nc.vector.tensor_add(T1[:], P[:, :, 0:16], P[:, :, 16:32])
                T2 = tpool.tile([128, 288, 8], fp16, tag="T2")
                nc.vector.tensor_add(T2[:], T1[:, :, 0:8], T1[:, :, 8:16])
                T3 = tpool.tile([128, 288, 4], fp16, tag="T3")
                nc.vector.tensor_add(T3[:], T2[:, :, 0:4], T2[:, :, 4:8])
                T4 = tpool.tile([128, 288, 2], fp16, tag="T4")
                nc.vector.tensor_add(T4[:], T3[:, :, 0:2], T3[:, :, 2:4])
                S = smalls.tile([128, 288], fp16, tag="S")
                nc.vector.tensor_add(S[:], T4[:, :, 0], T4[:, :, 1])

                # E = exp(S * SCALE) on ScalarE (fp16 in/out)
                E = smalls.tile([128, 288], fp16, tag="E")
                nc.scalar.activation(out=E[:], in_=S[:], func=Exp, scale=SCALE)

                # den[s,i] = sum_j E[s,i,j]  (fp32 accum)
                den = smalls.tile([128, 24], f32, tag="den")
                nc.vector.tensor_reduce(
                    out=den[:],
                    in_=E[:].rearrange("p (si j) -> p si j", j=12),
                    axis=X,
                    op=add,
                )
                rec = smalls.tile([128, 24], f32, tag="rec")
                with nc.allow_low_precision(
                    reason="softmax weights tolerate reduced precision"
                ):
                    nc.vector.reciprocal(out=rec[:], in_=den[:])
                    # rec16: sub-1 as-is, sub-2 folded with -lam
                    rec16 = smalls.tile([128, 24], fp16, tag="rec16")
                    nc.vector.tensor_copy(rec16[:, 0:12], rec[:, 0:12])
                    nc.vector.tensor_scalar(
                        out=rec16[:, 12:24],
                        in0=rec[:, 12:24],
                        scalar1=nlam_sb[:],
                        scalar2=None,
                        op0=mult,
                    )
                # Atilde = E1*rec1 + E2*(-lam*rec2)
                A = smalls.tile([128, 288], fp16, tag="A")
                nc.vector.tensor_mul(
                    A[:].rearrange("p (si j) -> p si j", j=12),
                    E[:].rearrange("p (si j) -> p si j", j=12),
                    rec16[:].unsqueeze(2).broadcast_to((128, 24, 12)),
                )
                At = smalls.tile([128, 144], fp16, tag="At")
                nc.vector.tensor_add(At[:], A[:, 0:144], A[:, 144:288])

                # ---- A.V contraction: y[i,c] = sum_j At[i,j] vT[c,j] ----
                # v^T view: [p, c(64), j(12)] contiguous inner j so both PO
                # operands keep stride-1 innermost (2x mode)
                vT = smalls.tile([128, 64, 12], fp16, tag="vT")
                nc.scalar.copy(
                    vT[:],
                    qkv[:, 2 * D : 3 * D].rearrange("p (j c) -> p c j", j=12),
                )
                PO = popool.tile([128, 12, 64, 12], fp16, tag="PO")
                nc.vector.tensor_mul(
                    PO[:],
                    At[:]
                    .rearrange("p (i j) -> p i j", i=12)
                    .unsqueeze(2)
                    .broadcast_to((128, 12, 64, 12)),
                    vT[:].unsqueeze(1).broadcast_to((128, 12, 64, 12)),
                )
                U1 = tpool.tile([128, 12, 64, 6], fp16, tag="U1")
                nc.vector.tensor_add(U1[:], PO[:, :, :, 0:6], PO[:, :, :, 6:12])
                U2a = tpool.tile([128, 12, 64, 2], fp16, tag="U2a")
                nc.vector.tensor_add(U2a[:], U1[:, :, :, 0:2], U1[:, :, :, 2:4])
                U2b = tpool.tile([128, 12, 64, 2], fp16, tag="U2b")
                nc.vector.tensor_add(U2b[:], U2a[:], U1[:, :, :, 4:6])
                y = ypool.tile([128, 768], fp16, tag="y")
                nc.vector.tensor_add(
                    y[:].rearrange("p (i c) -> p i c", i=12),
                    U2b[:, :, :, 0],
                    U2b[:, :, :, 1],
                )

                # ---- transpose y (PE) to feed GEMM2 stationary ----
                yT = ypool.tile([128, KC, 128], fp16, tag="yT")
                for c in range(KC):
                    tp = ps_tr.tile([128, 128], f32)
                    nc.tensor.transpose(
                        tp[:], y[:, c * 128 : (c + 1) * 128], ident[:]
                    )
                    nc.scalar.copy(yT[:, c, :], tp[:])

                # ---- GEMM2: out2 = y @ woT ----
                o2 = opool.tile([128, D], fp16)
                for ob, (o0, ow) in enumerate(WO_CH):
                    ps2 = ps_g2.tile([128, 512], f32)
                    for c in range(KC):
                        nc.tensor.matmul(
                            ps2[:, :ow],
                            yT[:, c, :],
                            wo_sbs[ob][:, c, :],
                            start=(c == 0),
                            stop=(c == KC - 1),
                        )
                    nc.scalar.copy(o2[:, o0 : o0 + ow], ps2[:, :ow])
                nc.sync.dma_start(out=out_d[t0 : t0 + TILE_T, :], in_=o2[:])

            def body():
                for it in range(ntiles):
                    emit_tile(it)

            if nrep == 1:
                body()
            else:
                with tc.For_i(0, nrep, 1):
                    body()

    nc.compile()
    return nc


def _get_program(ntok=TOK_PER_CORE, nrep=1):
    key = (ntok, nrep)
    if key not in _PROG_CACHE:
        _PROG_CACHE[key] = _build_program(ntok, nrep)
    return _PROG_CACHE[key]


def make_in_maps(x, W_attn, W_out, lambda_q1, lambda_k1, lambda_q2, lambda_k2):
    """Host-side packing: returns the per-core input dicts for the program."""
    x = np.asarray(x, dtype=np.float32)
    lam1 = np.exp(np.sum(np.float32(lambda_q1) * np.float32(lambda_k1)))
    lam2 = np.exp(np.sum(np.float32(lambda_q2) * np.float32(lambda_k2)))
    lam_full = np.float32(lam1 - lam2 + np.float32(LAMBDA_INIT))
    nlam = np.array([[-lam_full]], dtype=np.float32)

    waT = np.ascontiguousarray(
        np.asarray(W_attn, dtype=np.float32).T.astype(np.float16)
    )  # (768, 2304)
    woT = np.ascontiguousarray(
        (np.asarray(W_out, dtype=np.float32).T * np.float32(1.0 - LAMBDA_INIT))
        .astype(np.float16)
    )

    xf = x.reshape(TOK_TOTAL, D)
    ntiles = TOK_PER_CORE // TILE_T
    in_maps = []
    for c in range(N_CORES):
        xs = xf[c * TOK_PER_CORE : (c + 1) * TOK_PER_CORE]
        # xH[p, tile, c, ti] = xs[tile*128+ti, c*128+p]
        xh = np.ascontiguousarray(
            xs.reshape(ntiles, TILE_T, KC, 128)
            .transpose(3, 0, 2, 1)
            .astype(np.float16)
        )
        in_maps.append({"xH": xh, "waT": waT, "woT": woT, "nlam": nlam})
    return in_maps


def kernel(x, W_attn, W_out, lambda_q1, lambda_k1, lambda_q2, lambda_k2):
    in_maps = make_in_maps(
        x, W_attn, W_out, lambda_q1, lambda_k1, lambda_q2, lambda_k2
    )
    from concourse.bass_utils import run_bass_kernel_spmd

    nc = _get_program()
    res = run_bass_kernel_spmd(nc, in_maps, list(range(N_CORES)))
    outs = [res.results[i]["out"] for i in range(N_CORES)]
    y = np.concatenate(outs, axis=0).astype(np.float32).reshape(B, N, D)
    return y


# revision 8
# speedup vs baseline: 1.0650x; 1.0650x over previous
"""Trainium2 Bass kernel for nn_DifferentialFlashAttention.

Computation (per token t, fully parallel over the B*N = 32768 tokens):
  qkv = x @ W_attn.T ; split into q, k, v
  q/k split per head into two sub-vectors (q1, q2 / k1, k2) of 32 dims
  S_s[i,j] = q_s[i] . k_s[j] / sqrt(32)   (attention over the 12 HEADS)
  A_s = softmax_j(S_s); O_s[i] = sum_j A_s[i,j] * v[j]
  y = (O_1 - lam_full * O_2) * (1 - LAMBDA_INIT);  out = y @ W_out.T

Sharding: data-parallel over tokens across 8 NeuronCores (4096 tokens each).
Per core: tokens-on-partitions layout, 32 tiles of 128 tokens.

V2 design (all-fp16 datapath, PSUM accumulation in fp32):
  - GEMM1/GEMM2 fp16 operands on TensorE (stationary = transposed
    activation chunks, moving = pre-transposed fp16 weights in SBUF).
  - Per-token head-attention on VectorE in fp16 with every heavy
    tensor_tensor op shaped for the 2x DVE perf mode (innermost dims of
    both operands are real stride-1 runs, 4B-aligned; broadcasts only on
    outer dims).  Softmax combine y = (A1 - lam*A2).v folded before the
    A.V contraction so there is ONE product pass over (i,c,j).
  - Exp and all PSUM->SBUF copies on ScalarE.
  - The combine scale (1-LAMBDA_INIT) is folded into W_out on the host;
    -lam_full is folded into the sub-2 softmax reciprocal.
"""

import math
import sys

import numpy as np

if "/opt/trn_rl_repo" not in sys.path:
    sys.path.insert(0, "/opt/trn_rl_repo")

N_HEAD = 12
N_EMBD = 768
HEAD_DIM = 32  # per-sub head dim
DEPTH = 12
LAMBDA_INIT = 0.8 - 0.6 * math.exp(-0.3 * DEPTH)
B, N, D = 4, 8192, 768
N_CORES = 8
TOK_TOTAL = B * N
TOK_PER_CORE = TOK_TOTAL // N_CORES  # 4096
TILE_T = 128
KC = D // 128  # 6 contraction chunks
SCALE = 1.0 / math.sqrt(HEAD_DIM)

_PROG_CACHE = {}

# AV_PE: run the A.V contraction as 128 per-token [12x12]@[12x64] matmuls on
# TensorE (32x32 tile_position packing, 4 column-groups) instead of ~9.5K
# cycles/tile of fp16 broadcast-mul + tree-add on VectorE.  Needs two SBUF
# partition-crossing DMA relayouts per tile (v^T -> vB, At^T -> AtB) and one
# gather back to token-major y.
AV_PE = True


def _build_program(ntok, nrep=1):
    import concourse.bacc as bacc
    import concourse.tile as tile
    from concourse import mybir
    from concourse.masks import make_identity

    f32 = mybir.dt.float32
    fp16 = mybir.dt.float16
    Exp = mybir.ActivationFunctionType.Exp
    X = mybir.AxisListType.X
    add = mybir.AluOpType.add
    mult = mybir.AluOpType.mult

    ntiles = ntok // TILE_T

    nc = bacc.Bacc(
        "TRN2", target_bir_lowering=False, debug=False, num_devices=N_CORES
    )
    # x, host-packed fp16 so each partition's per-tile data is one contiguous
    # run: xH[p, tile, c, ti] = x[tile*128+ti, c*128+p]
    xH_d = nc.dram_tensor(
        "xH", [128, ntok // TILE_T, KC, TILE_T], fp16, kind="ExternalInput"
    ).ap()
    waT_d = nc.dram_tensor("waT", [D, 3 * D], fp16, kind="ExternalInput").ap()
    woT_d = nc.dram_tensor("woT", [D, D], fp16, kind="ExternalInput").ap()
    nlam_d = nc.dram_tensor("nlam", [1, 1], f32, kind="ExternalInput").ap()
    out_d = nc.dram_tensor("out", [ntok, D], fp16, kind="ExternalOutput").ap()

    with tile.TileContext(nc) as tc:
        from contextlib import ExitStack

        with ExitStack() as ctx:
            singles = ctx.enter_context(tc.tile_pool(name="singles", bufs=1))

            # ---- persistent weights / constants ----
            # weights split per output-chunk into separate tiles so tile 0's
            # first GEMM chunk only waits on its own slice of the preamble
            WA_CH = [(ob * 512, min(512, 3 * D - ob * 512)) for ob in range(5)]
            WO_CH = [(0, 512), (512, 256)]
            waT_r = waT_d.rearrange("(c p) o -> p c o", p=128)
            woT_r = woT_d.rearrange("(c p) o -> p c o", p=128)
            wa_sbs = [
                singles.tile([128, KC, ow], fp16, name=f"wa{ob}", tag=f"wa{ob}")
                for ob, (o0, ow) in enumerate(WA_CH)
            ]
            wo_sbs = [
                singles.tile([128, KC, ow], fp16, name=f"wo{ob}", tag=f"wo{ob}")
                for ob, (o0, ow) in enumerate(WO_CH)
            ]
            for (o0, ow), wa_t in zip(WA_CH, wa_sbs):
                nc.sync.dma_start(out=wa_t[:], in_=waT_r[:, :, o0 : o0 + ow])
            for (o0, ow), wo_t in zip(WO_CH, wo_sbs):
                nc.sync.dma_start(out=wo_t[:], in_=woT_r[:, :, o0 : o0 + ow])
            ident = singles.tile([128, 128], fp16)
            make_identity(nc, ident[:])
            nlam_sb = singles.tile([128, 1], f32)
            nc.gpsimd.dma_start(out=nlam_sb[:], in_=nlam_d.to_broadcast((128, 1)))

            xpool = ctx.enter_context(tc.tile_pool(name="xt", bufs=3))
            qkvpool = ctx.enter_context(tc.tile_pool(name="qkv", bufs=2))
            ppool = ctx.enter_context(tc.tile_pool(name="prod", bufs=1))
            tpool = ctx.enter_context(tc.tile_pool(name="tree", bufs=2))
            popool = ctx.enter_context(tc.tile_pool(name="oprod", bufs=1))
            smalls = ctx.enter_context(tc.tile_pool(name="smalls", bufs=2))
            ypool = ctx.enter_context(tc.tile_pool(name="y", bufs=2))
            opool = ctx.enter_context(tc.tile_pool(name="o2", bufs=2))
            ps_g1 = ctx.enter_context(
                tc.tile_pool(name="ps1", bufs=2 if AV_PE else 3, space="PSUM")
            )
            ps_tr = ctx.enter_context(tc.tile_pool(name="pstr", bufs=2, space="PSUM"))
            ps_g2 = ctx.enter_context(tc.tile_pool(name="ps2", bufs=2, space="PSUM"))
            if AV_PE:
                ps_att = ctx.enter_context(
                    tc.tile_pool(name="psatt", bufs=2, space="PSUM")
                )

            def emit_tile(it):
                t0 = it * TILE_T
                # ---- load x^T tile (stationary chunks for GEMM1) ----
                xt = xpool.tile([128, KC, TILE_T], fp16, tag="xt")
                nc.sync.dma_start(out=xt[:], in_=xH_d[:, it, :, :])

                # ---- GEMM1: qkv[t, 0:2304] ----
                qkv = qkvpool.tile([128, 3 * D], fp16)
                for ob, (o0, ow) in enumerate(WA_CH):
                    ps = ps_g1.tile([128, 512], f32)
                    for c in range(KC):
                        nc.tensor.matmul(
                            ps[:, :ow],
                            xt[:, c, :],
                            wa_sbs[ob][:, c, :],
                            start=(c == 0),
                            stop=(c == KC - 1),
                        )
                    nc.scalar.copy(qkv[:, o0 : o0 + ow], ps[:, :ow])

                # ---- attention over heads, per token ----
                # S[s,i,j] = sum_d q[s,i,d]*k[s,j,d]; one product per sub
                # (engine APs allow at most 3 free dims)
                P = ppool.tile([128, 288, 32], fp16, tag="P")
                for s in (0, 1):
                    qv = (
                        qkv[:, 0:D]
                        .rearrange("p (i s d) -> p s i d", i=12, s=2)[:, s]
                        .unsqueeze(2)
                        .broadcast_to((128, 12, 12, 32))
                    )
                    kv = (
                        qkv[:, D : 2 * D]
                        .rearrange("p (j s d) -> p s j d", j=12, s=2)[:, s]
                        .unsqueeze(1)
                        .broadcast_to((128, 12, 12, 32))
                    )
                    nc.vector.tensor_mul(
                        P[:, s * 144 : (s + 1) * 144, :].rearrange(
                            "p (i j) d -> p i j d", i=12
                        ),
                        qv,
                        kv,
                    )
                # tree-sum over d (fp16 tensor_tensor runs 2x; tensor_reduce 1x)
                T1 = tpool.tile([128, 288, 16], fp16, tag="T1")
                nc.vector.tensor_add(T1[:], P[:, :, 0:16], P[:, :, 16:32])
                T2 = tpool.tile([128, 288, 8], fp16, tag="T2")
                nc.vector.tensor_add(T2[:], T1[:, :, 0:8], T1[:, :, 8:16])
                T3 = tpool.tile([128, 288, 4], fp16, tag="T3")
                nc.vector.tensor_add(T3[:], T2[:, :, 0:4], T2[:, :, 4:8])
                T4 = tpool.tile([128, 288, 2], fp16, tag="T4")
                nc.vector.tensor_add(T4[:], T3[:, :, 0:2], T3[:, :, 2:4])
                S = smalls.tile([128, 288], fp16, tag="S")
                nc.vector.tensor_add(S[:], T4[:, :, 0], T4[:, :, 1])

                # E = exp(S * SCALE) on ScalarE (fp16 in/out)
                E = smalls.tile([128, 288], fp16, tag="E")
                nc.scalar.activation(out=E[:], in_=S[:], func=Exp, scale=SCALE)

                # den[s,i] = sum_j E[s,i,j]  (fp32 accum)
                den = smalls.tile([128, 24], f32, tag="den")
                nc.vector.tensor_reduce(
                    out=den[:],
                    in_=E[:].rearrange("p (si j) -> p si j", j=12),
                    axis=X,
                    op=add,
                )
                rec = smalls.tile([128, 24], f32, tag="rec")
                with nc.allow_low_precision(
                    reason="softmax weights tolerate reduced precision"
                ):
                    nc.vector.reciprocal(out=rec[:], in_=den[:])
                    # rec16: sub-1 as-is, sub-2 folded with -lam
                    rec16 = smalls.tile([128, 24], fp16, tag="rec16")
                    nc.vector.tensor_copy(rec16[:, 0:12], rec[:, 0:12])
                    nc.vector.tensor_scalar(
                        out=rec16[:, 12:24],
                        in0=rec[:, 12:24],
                        scalar1=nlam_sb[:],
                        scalar2=None,
                        op0=mult,
                    )
                # Atilde = E1*rec1 + E2*(-lam*rec2)
                A = smalls.tile([128, 288], fp16, tag="A")
                nc.vector.tensor_mul(
                    A[:].rearrange("p (si j) -> p si j", j=12),
                    E[:].rearrange("p (si j) -> p si j", j=12),
                    rec16[:].unsqueeze(2).broadcast_to((128, 24, 12)),
                )
                At = smalls.tile([128, 144], fp16, tag="At")
                nc.vector.tensor_add(At[:], A[:, 0:144], A[:, 144:288])

                if not AV_PE:
                    # ---- A.V on DVE: y[i,c] = sum_j At[i,j] vT[c,j] ----
                    # v^T view: [p, c(64), j(12)] contiguous inner j so both
                    # PO operands keep stride-1 innermost (2x mode)
                    vT = smalls.tile([128, 64, 12], fp16, tag="vT")
                    nc.scalar.copy(
                        vT[:],
                        qkv[:, 2 * D : 3 * D].rearrange("p (j c) -> p c j", j=12),
                    )
                    PO = popool.tile([128, 12, 64, 12], fp16, tag="PO")
                    nc.vector.tensor_mul(
                        PO[:],
                        At[:]
                        .rearrange("p (i j) -> p i j", i=12)
                        .unsqueeze(2)
                        .broadcast_to((128, 12, 64, 12)),
                        vT[:].unsqueeze(1).broadcast_to((128, 12, 64, 12)),
                    )
                    U1 = tpool.tile([128, 12, 64, 6], fp16, tag="U1")
                    nc.vector.tensor_add(
                        U1[:], PO[:, :, :, 0:6], PO[:, :, :, 6:12]
                    )
                    U2a = tpool.tile([128, 12, 64, 2], fp16, tag="U2a")
                    nc.vector.tensor_add(
                        U2a[:], U1[:, :, :, 0:2], U1[:, :, :, 2:4]
                    )
                    U2b = tpool.tile([128, 12, 64, 2], fp16, tag="U2b")
                    nc.vector.tensor_add(U2b[:], U2a[:], U1[:, :, :, 4:6])
                    y = ypool.tile([128, 768], fp16, tag="y")
                    nc.vector.tensor_add(
                        y[:].rearrange("p (i c) -> p i c", i=12),
                        U2b[:, :, :, 0],
                        U2b[:, :, :, 1],
                    )
                else:
                    # ---- A.V on TensorE: per-token O = At_t^T.T @ v_t ----
                    # (1) v^T via PE transposes: vTs[(par,c), ch, t] where the
                    #     v-feature j*64+c sits at chunk ch=j//2, row par*64+c,
                    #     par=j%2
                    vTs = ypool.tile([128, KC, 128], fp16, tag="vTs")
                    for c in range(KC):
                        tv = ps_tr.tile([128, 128], fp16)
                        nc.tensor.transpose(
                            tv[:],
                            qkv[:, 2 * D + c * 128 : 2 * D + (c + 1) * 128],
                            ident[:],
                        )
                        nc.scalar.copy(vTs[:, c, :], tv[:])
                    # (2) At^T via PE transposes: AtTs[(i6,j), h, t], half h
                    #     covers heads i = 6h + i6
                    AtTs = smalls.tile([72, 2, 128], fp16, tag="AtTs")
                    for h in (0, 1):
                        ta = ps_tr.tile([72, 128], fp16)
                        nc.tensor.transpose(
                            ta[:], At[:, h * 72 : (h + 1) * 72], ident[:]
                        )
                        nc.scalar.copy(AtTs[:, h, :], ta[:])
                    # (3) partition-crossing SBUF relayouts via DMA
                    vB = smalls.tile([12, 64, 128], fp16, tag="vB")
                    nc.sync.dma_start(
                        out=vB[:].rearrange("(ch par) c t -> par ch c t", ch=6),
                        in_=vTs[:].rearrange("(par c) ch t -> par ch c t", par=2),
                    )
                    AtB = smalls.tile([12, 12, 128], fp16, tag="AtB")
                    nc.sync.dma_start(
                        out=AtB[:].rearrange("j (h i6) t -> h i6 j t", h=2),
                        in_=AtTs[:].rearrange("(i6 j) h t -> h i6 j t", i6=6),
                    )
                    # (4) 128 per-token 32x32-tile matmuls; token t ->
                    #     col-group t%4, psum bank t//32, offset ((t//4)%8)*64
                    yP = ypool.tile([128, 4, 512], fp16, tag="yP")
                    for b in range(4):
                        pab = ps_att.tile([128, 512], f32, tag="pa")
                        for u in range(32):
                            t = b * 32 + u
                            cg = t % 4
                            off = (t // 4) % 8
                            nc.tensor.matmul(
                                pab[
                                    32 * cg : 32 * cg + 12,
                                    off * 64 : (off + 1) * 64,
                                ],
                                AtB[:, :, t],
                                vB[:, :, t],
                                start=True,
                                stop=True,
                            )
                        nc.scalar.copy(yP[:, b, :], pab[:])
                    # (5) gather back to token-major y[t, (i,c)]
                    y = ypool.tile([128, 768], fp16, tag="y")
                    nc.sync.dma_start(
                        out=y[:].rearrange(
                            "(b off cg) (i c) -> b off cg i c", b=4, off=8, i=12
                        ),
                        in_=yP[:].rearrange(
                            "(cg i) b (off c) -> b off cg i c", cg=4, off=8
                        ),
                    )

                # ---- transpose y (PE) to feed GEMM2 stationary ----
                yT = ypool.tile([128, KC, 128], fp16, tag="yT")
                for c in range(KC):
                    tp = ps_tr.tile([128, 128], fp16)
                    nc.tensor.transpose(
                        tp[:], y[:, c * 128 : (c + 1) * 128], ident[:]
                    )
                    nc.scalar.copy(yT[:, c, :], tp[:])

                # ---- GEMM2: out2 = y @ woT ----
                o2 = opool.tile([128, D], fp16)
                for ob, (o0, ow) in enumerate(WO_CH):
                    ps2 = ps_g2.tile([128, 512], f32)
                    for c in range(KC):
                        nc.tensor.matmul(
                            ps2[:, :ow],
                            yT[:, c, :],
                            wo_sbs[ob][:, c, :],
                            start=(c == 0),
                            stop=(c == KC - 1),
                        )
                    nc.scalar.copy(o2[:, o0 : o0 + ow], ps2[:, :ow])
                nc.sync.dma_start(out=out_d[t0 : t0 + TILE_T, :], in_=o2[:])

            def body():
                for it in range(ntiles):
                    emit_tile(it)

            if nrep == 1:
                body()
            else:
                with tc.For_i(0, nrep, 1):
                    body()

    nc.compile()
    return nc


def _get_program(ntok=TOK_PER_CORE, nrep=1):
    key = (ntok, nrep)
    if key not in _PROG_CACHE:
        _PROG_CACHE[key] = _build_program(ntok, nrep)
    return _PROG_CACHE[key]


def make_in_maps(x, W_attn, W_out, lambda_q1, lambda_k1, lambda_q2, lambda_k2):
    """Host-side packing: returns the per-core input dicts for the program."""
    x = np.asarray(x, dtype=np.float32)
    lam1 = np.exp(np.sum(np.float32(lambda_q1) * np.float32(lambda_k1)))
    lam2 = np.exp(np.sum(np.float32(lambda_q2) * np.float32(lambda_k2)))
    lam_full = np.float32(lam1 - lam2 + np.float32(LAMBDA_INIT))
    nlam = np.array([[-lam_full]], dtype=np.float32)

    waT = np.ascontiguousarray(
        np.asarray(W_attn, dtype=np.float32).T.astype(np.float16)
    )  # (768, 2304)
    woT = np.ascontiguousarray(
        (np.asarray(W_out, dtype=np.float32).T * np.float32(1.0 - LAMBDA_INIT))
        .astype(np.float16)
    )

    xf = x.reshape(TOK_TOTAL, D)
    ntiles = TOK_PER_CORE // TILE_T
    in_maps = []
    for c in range(N_CORES):
        xs = xf[c * TOK_PER_CORE : (c + 1) * TOK_PER_CORE]
        # xH[p, tile, c, ti] = xs[tile*128+ti, c*128+p]
        xh = np.ascontiguousarray(
            xs.reshape(ntiles, TILE_T, KC, 128)
            .transpose(3, 0, 2, 1)
            .astype(np.float16)
        )
        in_maps.append({"xH": xh, "waT": waT, "woT": woT, "nlam": nlam})
    return in_maps


def kernel(x, W_attn, W_out, lambda_q1, lambda_k1, lambda_q2, lambda_k2):
    in_maps = make_in_maps(
        x, W_attn, W_out, lambda_q1, lambda_k1, lambda_q2, lambda_k2
    )
    from concourse.bass_utils import run_bass_kernel_spmd

    nc = _get_program()
    res = run_bass_kernel_spmd(nc, in_maps, list(range(N_CORES)))
    outs = [res.results[i]["out"] for i in range(N_CORES)]
    y = np.concatenate(outs, axis=0).astype(np.float32).reshape(B, N, D)
    return y
